# revision 16
# baseline (speedup 1.0000x reference)
"""Grouped-GEMM MoE experts (E=64, H=2048, F=1408, 16 tokens/expert, SwiGLU),
expert-parallel across 8 Trainium2 NeuronCores.

Memory-bound kernel: per core the 3 weight tensors are the traffic. Two host-
side tricks halve + streamline it:
  1. Weights are cast to bf16 on host (rel-err ~1e-3, tolerance is 2e-2);
     halves HBM traffic AND runs matmuls at 1 cycle/row instead of fp32's 4.
  2. Weights are pre-packed into the exact SBUF tile layout the kernel
     consumes: uniform [128, 11264] units, fully contiguous, so every weight
     DMA is a single 2.88 MB line-rate transfer. x is pre-transposed too.

Per-expert unit layout (6 units of 11264 cols):
  units 0-3: 4 h-chunks each of interleaved (w1 | w3) [128h, 1408f] blocks
  units 4-5: w2 packed [128f, fc, 2048h] split at col 11264 (512 | 11264)

Compute (unchanged from the proven fp32 version, bf16 dtypes):
  gateT/upT [f,tok] = W1/W3 chunk.T @ xT chunk   (weight-stationary, FWL)
  interT = silu(gateT) * upT                      (bf16, [128, 176])
  out[tok,h]  = interT chunk.T @ W2 chunk         (inter-stationary, N=512)
"""

import sys

if "/opt/trn_rl_repo" not in sys.path:
    sys.path.insert(0, "/opt/trn_rl_repo")

import numpy as np
import ml_dtypes

E, H, F = 64, 2048, 1408
TOK = 16                  # tokens per expert (uniform routing)
NCORES = 8
E_LOC = E // NCORES       # 8 experts per core
T_LOC = E_LOC * TOK       # 128 tokens per core
P = 128
HC = H // P               # 16 contraction chunks for gate/up
FC = F // P               # 11 contraction chunks for down
UCOLS = 2 * F * 4         # 11264 cols per weight unit
NU = 6                    # units per expert: 4 gate/up + 2 down
NFREE = 512               # matmul max free dim = one PSUM bank
BF16 = ml_dtypes.bfloat16

_cache = {}


def _build_nc():
    import concourse.mybir as mybir
    from concourse import bacc

    from concourse.tile import TileContext

    f32 = mybir.dt.float32
    bf16 = mybir.dt.bfloat16
    AF = mybir.ActivationFunctionType

    nc = bacc.Bacc()
    xt_d = nc.declare_dram_parameter("xt", [P, HC * T_LOC], bf16, isOutput=False)
    w_d = nc.declare_dram_parameter(
        "w", [E_LOC, 3, P, 2 * UCOLS], bf16, isOutput=False
    )
    y_d = nc.declare_dram_parameter("y", [T_LOC, H], f32, isOutput=True)
    PC = 2 * UCOLS  # 22528 cols per DMA pair: (gu01 | gu23 | dn45)
    CW = 2 * F      # 2816 cols per gate/up h-chunk

    with TileContext(nc) as tc:
        with (
            tc.tile_pool(name="xs", bufs=1) as xs,
            tc.tile_pool(name="wt", bufs=4) as wt,
            tc.tile_pool(name="acts", bufs=2) as acts,
            tc.tile_pool(name="ps_gu", bufs=2, space="PSUM") as ps_gu,
            tc.tile_pool(name="ps_dn", bufs=1, space="PSUM") as ps_dn,
        ):
            # xt on the scalar HWDGE ring so the weight stream (sync ring)
            # starts immediately; xt is only needed ~15us in.
            xt = xs.tile([P, HC * T_LOC], bf16)
            nc.scalar.dma_start(out=xt[:], in_=xt_d[:, :])

            for e in range(E_LOC):
                last = e == E_LOC - 1
                # weight stream: 3 big DMAs per expert (gu01 | gu23 | dn45).
                # For the last expert, stream the tail in shrinking pieces
                # whose arrival matches the compute chain, so PE idle slivers
                # stay under the ~3.4us HAM window and almost no compute
                # remains after the last weight byte.
                gu_map = []   # h-chunk c -> (tile, col base)
                units = []    # (tile, global dn col base, end)
                if not last:
                    for pair in range(2):
                        t = wt.tile([P, PC], bf16, tag="w")
                        nc.sync.dma_start(out=t[:], in_=w_d[e, pair, :, :])
                        gu_map += [(t, cs * CW) for cs in range(8)]
                    t = wt.tile([P, PC], bf16, tag="w")
                    nc.sync.dma_start(out=t[:], in_=w_d[e, 2, :, :])
                    units.append((t, 0, PC))
                else:
                    t = wt.tile([P, PC], bf16, tag="w")
                    nc.sync.dma_start(out=t[:], in_=w_d[e, 0, :, :])
                    gu_map += [(t, cs * CW) for cs in range(8)]
                    # gu chunks 8-11, then 12-13, 14, 15 as pieces
                    for lo, hi in ((0, 4), (4, 6), (6, 7), (7, 8)):
                        t = wt.tile([P, PC], bf16, tag="w")
                        nc.sync.dma_start(
                            out=t[:, : (hi - lo) * CW],
                            in_=w_d[e, 1, :, lo * CW : hi * CW],
                        )
                        gu_map += [(t, cs * CW) for cs in range(hi - lo)]
                    # dn pieces at f-chunk granularity: fc0-1, 2-4, 5-7, 8-9, 10
                    for lo, hi in ((0, 2), (2, 5), (5, 8), (8, 10), (10, 11)):
                        t = wt.tile([P, PC], bf16, tag="w")
                        nc.sync.dma_start(
                            out=t[:, : (hi - lo) * H],
                            in_=w_d[e, 2, :, lo * H : hi * H],
                        )
                        units.append((t, lo * H, hi * H))

                # gate/up: all FC output chunks share one PSUM bank per
                # tensor; only the first matmul into the bank clears it
                # (start=True), later chunks overwrite via has_written.
                gt = ps_gu.tile([P, FC * TOK], f32, tag="gt")
                ut = ps_gu.tile([P, FC * TOK], f32, tag="ut")
                rhs_e = e * TOK
                for c in range(HC):
                    wu, base = gu_map[c]
                    rhs = xt[:, c * T_LOC + rhs_e : c * T_LOC + rhs_e + TOK]
                    first = c == 0
                    final = c == HC - 1
                    w1o = base
                    w3o = base + F
                    for fc in range(FC):
                        nc.tensor.matmul(
                            gt[:, fc * TOK : (fc + 1) * TOK],
                            wu[:, w1o + fc * P : w1o + (fc + 1) * P],
                            rhs,
                            start=(first and fc == 0),
                            stop=(final and fc == FC - 1),
                            skip_group_check=True,
                        )
                    for fc in range(FC):
                        nc.tensor.matmul(
                            ut[:, fc * TOK : (fc + 1) * TOK],
                            wu[:, w3o + fc * P : w3o + (fc + 1) * P],
                            rhs,
                            start=(first and fc == 0),
                            stop=(final and fc == FC - 1),
                            skip_group_check=True,
                        )

                gs = acts.tile([P, FC * TOK], f32, tag="gs")
                it = acts.tile([P, FC * TOK], bf16, tag="it")
                nc.scalar.activation(gs[:], gt[:], AF.Silu)
                nc.vector.tensor_mul(it[:], gs[:], ut[:])

                def dn_src(col):
                    for t, lo, hi in units:
                        if lo <= col < hi:
                            return t, col - lo
                    raise AssertionError

                dn = ps_dn.tile([P, H], f32, tag="dn")
                for fc in range(FC):
                    for nt in range(H // NFREE):
                        col = fc * H + nt * NFREE
                        wu, off = dn_src(col)
                        nc.tensor.matmul(
                            dn[:TOK, nt * NFREE : (nt + 1) * NFREE],
                            it[:, fc * TOK : (fc + 1) * TOK],
                            wu[:, off : off + NFREE],
                            start=(fc == 0),
                            stop=(fc == FC - 1),
                        )

                # copy out per 512-col slice (PSUM bank granularity) so the
                # copies overlap the last f-chunk's matmuls, then 2 half-row
                # output DMAs; shortens the after-last-matmul critical path.
                ob = acts.tile([TOK, H], f32, tag="ob")
                for nt in range(H // NFREE):
                    nc.vector.tensor_copy(
                        out=ob[:, nt * NFREE : (nt + 1) * NFREE],
                        in_=dn[:TOK, nt * NFREE : (nt + 1) * NFREE],
                    )
                # y goes out on the scalar HWDGE ring: a y DMA's sem-wait on
                # the ob copies must never stall the sync ring's weight stream.
                rows = slice(e * TOK, (e + 1) * TOK)
                nc.scalar.dma_start(out=y_d[rows, : H // 2], in_=ob[:, : H // 2])
                nc.scalar.dma_start(out=y_d[rows, H // 2 :], in_=ob[:, H // 2 :])

    if not nc.is_finalized():
        nc.finalize()
    return nc


def _get_nc():
    if "nc" not in _cache:
        _cache["nc"] = _build_nc()
    return _cache["nc"]


def _pack_core(x, w1, w3, w2):
    """Pack one core's slice into the kernel's DMA-ready bf16 layout."""
    # xT: [p, c*T_LOC + t] = x[t, c*128 + p]
    xt = np.ascontiguousarray(
        x.reshape(T_LOC, HC, P).transpose(2, 1, 0).reshape(P, HC * T_LOC)
    ).astype(BF16)
    # gate/up units: [e, u, p, (cs, {w1,w3}, f)]
    w1r = w1.reshape(E_LOC, HC, P, F)
    w3r = w3.reshape(E_LOC, HC, P, F)
    gu = np.stack([w1r, w3r], axis=3)               # [e, c, p, s, f]
    gu = gu.reshape(E_LOC, 4, 4, P, 2, F)           # [e, u, cs, p, s, f]
    gu = gu.transpose(0, 1, 3, 2, 4, 5).reshape(E_LOC, 4, P, UCOLS)
    # down units: [e, p, fc*H + h] split into 2 units of UCOLS
    dn = w2.reshape(E_LOC, FC, P, H).transpose(0, 2, 1, 3).reshape(E_LOC, P, 2, UCOLS)
    dn = dn.transpose(0, 2, 1, 3)                   # [e, 2, p, UCOLS]
    w = np.concatenate([gu, dn], axis=1).astype(BF16)  # [e, 6, p, UCOLS]
    # merge unit pairs so each is one contiguous [128, 2*UCOLS] DMA:
    # (gu01 | gu23 | dn45)
    w = np.ascontiguousarray(
        w.reshape(E_LOC, 3, 2, P, UCOLS)
        .transpose(0, 1, 3, 2, 4)
        .reshape(E_LOC, 3, P, 2 * UCOLS)
    )
    return xt, w


def _make_in_maps(inputs):
    x = np.asarray(inputs["permuted_local_hidden_states"], dtype=np.float32)
    w1 = np.asarray(inputs["gate_proj"], dtype=np.float32)
    w3 = np.asarray(inputs["up_proj"], dtype=np.float32)
    w2 = np.asarray(inputs["down_proj"], dtype=np.float32)
    in_maps = []
    for m in range(NCORES):
        xt, w = _pack_core(
            x[m * T_LOC : (m + 1) * T_LOC],
            w1[m * E_LOC : (m + 1) * E_LOC],
            w3[m * E_LOC : (m + 1) * E_LOC],
            w2[m * E_LOC : (m + 1) * E_LOC],
        )
        in_maps.append({"xt": xt, "w": w})
    return in_maps


def run(inputs, trace=False, **kwargs):
    """Run the SPMD kernel; returns (y_full, BassKernelResults)."""
    from concourse.bass_utils import run_bass_kernel_spmd

    nc = _get_nc()
    res = run_bass_kernel_spmd(
        nc, _make_in_maps(inputs), list(range(NCORES)), trace=trace, **kwargs
    )
    y = np.concatenate([res.results[m]["y"] for m in range(NCORES)], axis=0)
    return y.astype(np.float32, copy=False), res


def kernel(**inputs):
    y, _ = run(inputs, trace=False)
    return y
